# revision 2
# baseline (speedup 1.0000x reference)
"""GNN message-passing (3x GraphConv + mean-pool of graph 0 + FC + softmax)
on 8 Trainium2 NeuronCores.

Strategy (v2)
-------------
Nodes are interleave-partitioned across 8 cores (graph-0 nodes first, then
the 2-hop frontier, then layer-1-send nodes, then the rest, each degree-
sorted for balance). Aggregation is matmul-based: gathered source rows
[128 edges, d] are multiplied with one-hot destination-select matrices
(built on the Vector engine via a broadcast is_equal against an iota row)
and accumulated per 128-destination chunk in PSUM. No slot-grid padding
and no vector-engine tree fold.

- Layer 1 gathers x rows directly (bf16, 256B descriptors) from a
  replicated permuted node table -- no z0 table and no AllGather.
- Layers 2/3 exchange per-edge z rows via AllToAll: each core gathers
  z[src] rows from its local z table into per-peer blocks (cast to bf16
  on the way out), AllToAll delivers them, and the receiver loads its
  recv buffer with plain large DMAs -- the rows arrive pre-sorted by
  destination chunk and feed the select-matmuls directly.
- Only probs[0] is returned, so layers 2/3 are pruned to the 1-hop/2-hop
  in-neighborhoods of graph-0 nodes (exact, computed from the inputs).
- Mean-pool partials are AllReduce-d; FC + softmax run redundantly.
"""

import numpy as np

import concourse.bacc as bacc
import concourse.bass as bass
import concourse.mybir as mybir
import concourse.tile as tile
from concourse._compat import cdiv
from concourse.bass_utils import run_bass_kernel_spmd

NCORES = 8
LO = 32768
F32 = mybir.dt.float32
BF16 = mybir.dt.bfloat16
I16 = mybir.dt.int16
AX = mybir.AluOpType
ACTF = mybir.ActivationFunctionType


class Plan:
    pass


def _wrap_idx(flat):
    w = np.asarray(flat, np.int16).reshape(-1, 16).T.copy()
    return np.tile(w, (8, 1))


def build_plan(x, edge_index, batch):
    import ml_dtypes
    p = Plan()
    N, F = x.shape
    src = np.asarray(edge_index[0], dtype=np.int64)
    dst = np.asarray(edge_index[1], dtype=np.int64)
    batch = np.asarray(batch, dtype=np.int64)
    NPC = cdiv(N, NCORES)
    NLOC = cdiv(NPC + 1, 128) * 128
    p.N, p.F, p.NPC, p.NLOC = N, F, NPC, NLOC

    in_T0 = batch == 0
    p.n0 = int(in_T0.sum())
    e3 = in_T0[dst]
    in_T2 = in_T0.copy()
    in_T2[src[e3]] = True
    e2 = in_T2[dst]
    in_W1 = np.zeros(N, bool)
    in_W1[src[e2]] = True

    deg1 = np.bincount(dst, minlength=N)
    deg2 = np.bincount(dst[e2], minlength=N)
    deg3 = np.bincount(dst[e3], minlength=N)

    nodes = np.arange(N)
    g0 = nodes[in_T0]
    g1 = nodes[in_T2 & ~in_T0]
    g2 = nodes[in_W1 & ~in_T2]
    g3 = nodes[~in_T2 & ~in_W1]
    g0 = g0[np.argsort(-deg3[g0], kind="stable")]
    g1 = g1[np.argsort(-deg2[g1], kind="stable")]
    g2 = g2[np.argsort(-deg1[g2], kind="stable")]
    g3 = g3[np.argsort(-deg1[g3], kind="stable")]
    order = np.concatenate([g0, g1, g2, g3])
    j = np.arange(N)
    node_core = np.empty(N, np.int64)
    pos = np.empty(N, np.int64)
    node_core[order] = j % NCORES
    pos[order] = j // NCORES
    n0_k = np.bincount(j[:len(g0)] % NCORES, minlength=NCORES)
    n2_k = np.bincount(j[:len(g0) + len(g1)] % NCORES, minlength=NCORES)
    nw_k = np.bincount(j[:len(g0) + len(g1) + len(g2)] % NCORES,
                       minlength=NCORES)
    p.pos, p.node_core = pos, node_core
    C1 = NLOC // 128
    C2 = max(1, cdiv(int(n2_k.max()), 128))
    C3 = max(1, cdiv(int(n0_k.max()), 128))
    WZ1 = max(max(1, cdiv(int(nw_k.max()), 128)), C2)
    p.C = [C1, C2, C3]
    p.WZ1 = WZ1
    p.n0_k = n0_k

    core_of = node_core[dst]
    srcpos_all = node_core[src] * NLOC + pos[src]

    # ---- layer 1 tile stream (dense, chunk-aligned, lo/hi split) ----
    def layer0_stream(gcap=48):
        col = pos[dst]
        chunk = col // 128
        hi = srcpos_all >= LO
        Klo = np.zeros(C1, np.int64)
        Khi = np.zeros(C1, np.int64)
        for k in range(NCORES):
            m = core_of == k
            c_lo = np.bincount(chunk[m & ~hi], minlength=C1)
            c_hi = np.bincount(chunk[m & hi], minlength=C1)
            Klo = np.maximum(Klo, -(-c_lo // 128))
            Khi = np.maximum(Khi, -(-c_hi // 128))
        Klo = np.maximum(Klo, 1)
        groups = []
        cur, cur_t = [], 0
        for c in range(C1):
            t = int(Klo[c] + Khi[c])
            if cur and cur_t + t > gcap:
                groups.append(cur)
                cur, cur_t = [], 0
            cur.append(c)
            cur_t += t
        if cur:
            groups.append(cur)
        lo_off, hi_off = {}, {}
        off = 0
        gmeta = []
        for grp in groups:
            g_off = off
            for c in grp:
                lo_off[c] = off
                off += int(Klo[c])
            nlo = off - g_off
            for c in grp:
                hi_off[c] = off
                off += int(Khi[c])
            gmeta.append((g_off, nlo, off - g_off - nlo))
        NT = off
        sched = []
        for c in range(C1):
            tiles = (list(range(lo_off[c], lo_off[c] + int(Klo[c]))) +
                     list(range(hi_off[c], hi_off[c] + int(Khi[c]))))
            sched.append([(t, t) for t in tiles])
        idxs, dvals = [], []
        for k in range(NCORES):
            m = core_of == k
            flat = np.zeros(NT * 128, np.int64)
            dv = np.full(NT * 128, -1.0, np.float32)
            for c in range(C1):
                for half, offm in ((False, lo_off), (True, hi_off)):
                    em = m & (chunk == c) & (hi == half)
                    sp = srcpos_all[em] - (LO if half else 0)
                    cl = pos[dst[em]] % 128
                    o = offm[c] * 128
                    flat[o:o + len(sp)] = sp
                    dv[o:o + len(sp)] = cl
            idxs.append(flat)
            dvals.append(dv)
        return dict(NT=NT, groups=gmeta, sched=sched, idxs=idxs, dvals=dvals)

    p.L0 = layer0_stream()

    # ---- layers 2/3: AllToAll per-edge exchange ----
    def exchange_layer(emask, CL):
        es, ed = src[emask], dst[emask]
        ecore = core_of[emask]
        score = node_core[es]
        col = pos[ed]
        chunk = col // 128
        cnt = np.zeros((NCORES, NCORES), np.int64)
        for m in range(NCORES):
            for k in range(NCORES):
                cnt[m, k] = int(((score == m) & (ecore == k)).sum())
        S = max(cdiv(int(cnt.max()), 128) * 128, 128)
        nr = S // 128
        pair_set = set()
        per_core = []
        for k in range(NCORES):
            mk = ecore == k
            o = np.lexsort((col[mk], chunk[mk], score[mk]))
            es_k = es[mk][o]
            sc_k = score[mk][o]
            ch_k = chunk[mk][o]
            cl_k = (col[mk] % 128)[o]
            per_core.append((es_k, sc_k, ch_k, cl_k))
            for m in range(NCORES):
                bm = sc_k == m
                ch_b = ch_k[bm]
                t_local = np.arange(len(ch_b)) // 128
                for t, c in set(zip((m * nr + t_local).tolist(),
                                    ch_b.tolist())):
                    pair_set.add((t, c))
        pairs = sorted(pair_set, key=lambda tc: (tc[1], tc[0]))
        dvcol_of = {pc: i for i, pc in enumerate(pairs)}
        NDV = max(len(pairs), 1)
        sched = [[] for _ in range(CL)]
        for (t, c), i in dvcol_of.items():
            sched[c].append((t, i))
        for c in range(CL):
            sched[c].sort(key=lambda ti: ti[1])
        dvals, send_idx = [], []
        for k in range(NCORES):
            es_k, sc_k, ch_k, cl_k = per_core[k]
            dv = np.full(NDV * 128, -1.0, np.float32)
            for m in range(NCORES):
                bm = sc_k == m
                ch_b = ch_k[bm]
                cl_b = cl_k[bm]
                jj = np.arange(len(ch_b))
                tt = m * nr + jj // 128
                qq = jj % 128
                for t, c, q, cl in zip(tt.tolist(), ch_b.tolist(),
                                       qq.tolist(), cl_b.tolist()):
                    dv[dvcol_of[(t, c)] * 128 + q] = cl
            dvals.append(dv)
            sidx = np.zeros(NCORES * S, np.int64)
            for kk in range(NCORES):
                es_r, sc_r, _, _ = per_core[kk]
                ps = pos[es_r[sc_r == k]]
                sidx[kk * S:kk * S + len(ps)] = ps
            send_idx.append(sidx)
        return dict(S=S, NT=NCORES * nr, NDV=NDV, sched=sched, dvals=dvals,
                    send_idx=send_idx)

    p.L1 = exchange_layer(e2, C2)
    p.L2 = exchange_layer(e3, C3)

    xbf = np.asarray(x, np.float32).astype(ml_dtypes.bfloat16)
    p.x_tab = np.zeros((NCORES * NLOC, F), ml_dtypes.bfloat16)
    p.x_tab[node_core * NLOC + pos] = xbf
    p.xT = []
    for k in range(NCORES):
        xp = np.zeros((NLOC, F), ml_dtypes.bfloat16)
        kn = nodes[node_core == k]
        xp[pos[kn]] = xbf[kn]
        p.xT.append(np.ascontiguousarray(xp.T))

    p.idx0 = [_wrap_idx(v) for v in p.L0["idxs"]]
    p.dv0 = [np.ascontiguousarray(
        v.reshape(-1, 128).T.astype(ml_dtypes.bfloat16))
        for v in p.L0["dvals"]]
    p.sidx1 = [_wrap_idx(v) for v in p.L1["send_idx"]]
    p.dv1 = [np.ascontiguousarray(
        v.reshape(-1, 128).T.astype(ml_dtypes.bfloat16))
        for v in p.L1["dvals"]]
    p.sidx2 = [_wrap_idx(v) for v in p.L2["send_idx"]]
    p.dv2 = [np.ascontiguousarray(
        v.reshape(-1, 128).T.astype(ml_dtypes.bfloat16))
        for v in p.L2["dvals"]]

    p.mask = []
    for k in range(NCORES):
        m = np.zeros(C3 * 128, np.float32)
        m[:n0_k[k]] = 1.0
        p.mask.append(np.broadcast_to(
            m.astype(ml_dtypes.bfloat16), (64, C3 * 128)).copy())
    p.iota = np.ascontiguousarray(
        (np.arange(128, dtype=np.float32).reshape(1, 1, 128) *
         np.ones((128, 1, 1), np.float32)).astype(ml_dtypes.bfloat16))
    return p


def build_program(p, skip_collectives=False, repeat=1):
    nc = bacc.Bacc("TRN2")
    NLOC, F = p.NLOC, p.F
    C1, C2, C3 = p.C
    L0, L1, L2 = p.L0, p.L1, p.L2
    SELB = 48

    xtab_d = nc.dram_tensor("x_tab", [NCORES * NLOC, F], BF16,
                            kind="ExternalInput")
    xT_d = nc.dram_tensor("xT", [F, NLOC], BF16, kind="ExternalInput")
    idx0_d = nc.dram_tensor("idx0", [128, L0["NT"] * 8], I16,
                            kind="ExternalInput")
    dv0_d = nc.dram_tensor("dv0", [128, L0["NT"]], BF16,
                           kind="ExternalInput")
    sidx1_d = nc.dram_tensor("sidx1", [128, NCORES * L1["S"] // 16], I16,
                             kind="ExternalInput")
    dv1_d = nc.dram_tensor("dv1", [128, L1["NDV"]], BF16,
                           kind="ExternalInput")
    sidx2_d = nc.dram_tensor("sidx2", [128, NCORES * L2["S"] // 16], I16,
                             kind="ExternalInput")
    dv2_d = nc.dram_tensor("dv2", [128, L2["NDV"]], BF16,
                           kind="ExternalInput")
    iota_d = nc.dram_tensor("iota", [128, 1, 128], BF16,
                            kind="ExternalInput")
    mask_d = nc.dram_tensor("mask", [64, C3 * 128], BF16,
                            kind="ExternalInput")
    w_d = {}
    for nm, shp in [("W_rel1", [F, 64]), ("W_root1", [F, 64]),
                    ("W_rel2", [64, 64]), ("W_root2", [64, 64]),
                    ("W_rel3", [64, 64]), ("W_root3", [64, 64])]:
        w_d[nm] = nc.dram_tensor(nm, shp, BF16, kind="ExternalInput")
    wfc_d = nc.dram_tensor("W_fc", [64, 10], F32, kind="ExternalInput")
    bfc_d = nc.dram_tensor("b_fc", [1, 10], F32, kind="ExternalInput")
    out_d = nc.dram_tensor("probs", [1, 10], F32, kind="ExternalOutput")
    rg = [list(range(NCORES))]

    with tile.TileContext(nc) as tc:
        with (
            tc.tile_pool(name="const", bufs=1) as cpool,
            tc.tile_pool(name="persist", bufs=1) as ppool,
            tc.tile_pool(name="stream", bufs=3) as spool,
            tc.tile_pool(name="gather", bufs=2) as gpool,
            tc.tile_pool(name="sel", bufs=2) as selpool,
            tc.tile_pool(name="psum", bufs=1, space="PSUM") as psum,
            tc.tile_pool(name="dram", bufs=1, space="DRAM") as dram,
        ):
            w_s = {}
            for nm in w_d:
                shp = [F, 64] if nm in ("W_rel1", "W_root1") else [64, 64]
                w_s[nm] = cpool.tile(shp, BF16, tag=nm, name=nm)
                nc.sync.dma_start(w_s[nm][:], w_d[nm][:])
            wfc_s = cpool.tile([64, 10], F32, tag="wfc")
            nc.sync.dma_start(wfc_s[:], wfc_d[:])
            bfc_s = cpool.tile([1, 10], F32, tag="bfc")
            nc.sync.dma_start(bfc_s[:], bfc_d[:])
            mask_s = cpool.tile([64, C3 * 128], BF16, tag="mask")
            nc.sync.dma_start(mask_s[:], mask_d[:])
            iota_s = cpool.tile([128, 1, 128], BF16, tag="iota")
            nc.sync.dma_start(iota_s[:], iota_d[:])
            xT_s = cpool.tile([F, NLOC], BF16, tag="xT")
            nc.sync.dma_start(xT_s[:], xT_d[:])
            idx0_s = cpool.tile([128, L0["NT"] * 8], I16, tag="idx0")
            nc.sync.dma_start(idx0_s[:], idx0_d[:])
            dv0_s = cpool.tile([128, L0["NT"], 1], BF16, tag="dv0")
            nc.sync.dma_start(dv0_s[:], dv0_d[:])
            sidx1_s = cpool.tile([128, NCORES * L1["S"] // 16], I16,
                                 tag="sidx1")
            nc.sync.dma_start(sidx1_s[:], sidx1_d[:])
            dv1_s = cpool.tile([128, L1["NDV"], 1], BF16, tag="dv1")
            nc.sync.dma_start(dv1_s[:], dv1_d[:])
            sidx2_s = cpool.tile([128, NCORES * L2["S"] // 16], I16,
                                 tag="sidx2")
            nc.sync.dma_start(sidx2_s[:], sidx2_d[:])
            dv2_s = cpool.tile([128, L2["NDV"], 1], BF16, tag="dv2")
            nc.sync.dma_start(dv2_s[:], dv2_d[:])
            zero_s = cpool.tile([1, 64], BF16, tag="zero")
            nc.vector.memset(zero_s[:], 0.0)

            hT = [ppool.tile([64, NLOC], BF16, tag="h1T", name="h1T"),
                  ppool.tile([64, C2 * 128], BF16, tag="h2T", name="h2T"),
                  ppool.tile([64, C3 * 128], BF16, tag="h3T", name="h3T")]

            def build_sel(dv_s, t0, nt, selt):
                nc.vector.tensor_tensor(
                    out=selt[:, 0:nt, :],
                    in0=dv_s[:, t0:t0 + nt, :].to_broadcast([128, nt, 128]),
                    in1=iota_s[:].to_broadcast([128, nt, 128]),
                    op=AX.is_equal)

            for _rep in range(repeat):
                z1_own = dram.tile([NLOC, 64], F32, name=f"z1o_{_rep}")
                z2_own = dram.tile([C2 * 128, 64], F32, name=f"z2o_{_rep}")
                send1 = dram.tile([NCORES, 128, L1["S"] // 128, 64], BF16,
                                  name=f"send1_{_rep}")
                recv1 = dram.tile([NCORES, 128, L1["S"] // 128, 64], BF16,
                                  name=f"recv1_{_rep}")
                send2 = dram.tile([NCORES, 128, L2["S"] // 128, 64], BF16,
                                  name=f"send2_{_rep}")
                recv2 = dram.tile([NCORES, 128, L2["S"] // 128, 64], BF16,
                                  name=f"recv2_{_rep}")
                pool_in = dram.tile([64, 1], F32, name=f"pool_in_{_rep}")
                pool_out = dram.tile([64, 1], F32, addr_space="Shared",
                                     name=f"pool_out_{_rep}")

                def send_layer(li):
                    Lx, z_own = (L1, z1_own) if li == 1 else (L2, z2_own)
                    zrows = p.WZ1 * 128 if li == 1 else C2 * 128
                    sidx = sidx1_s if li == 1 else sidx2_s
                    sbuf_d = send1 if li == 1 else send2
                    rbuf_d = recv1 if li == 1 else recv2
                    S = Lx["S"]
                    nr = S // 128
                    for m in range(NCORES):
                        sg = spool.tile([128, nr, 64], F32, tag=f"sg{li}",
                                        name=f"sg{li}")
                        nc.gpsimd.dma_gather(
                            sg[:], z_own[0:zrows, :],
                            sidx[:, m * S // 16:(m + 1) * S // 16],
                            S, S, 64, single_packet=False)
                        nc.gpsimd.dma_start(sbuf_d[m], sg[:])
                    if not skip_collectives:
                        nc.gpsimd.collective_compute(
                            "AllToAll", AX.bypass, replica_groups=rg,
                            ins=[sbuf_d.opt()], outs=[rbuf_d.opt()])

                # ---------- layer 1 ----------
                def l0_chunk(c, Gf, self0):
                    sched = L0["sched"][c]
                    aggp = psum.tile([128, 128], F32, tag="aggX", bufs=2)
                    for i, (t, dvc) in enumerate(sched):
                        nc.tensor.matmul(
                            aggp[:], lhsT=Gf(t), rhs=self0(dvc),
                            start=(i == 0), stop=(i == len(sched) - 1))
                    aggs = spool.tile([128, 128], BF16, tag="aggXs")
                    nc.scalar.activation(aggs[:], aggp[:], ACTF.Copy)
                    hp = psum.tile([64, 128], F32, tag="hp", bufs=2)
                    sl = slice(c * 128, (c + 1) * 128)
                    nc.tensor.matmul(hp[:], lhsT=w_s["W_rel1"][:],
                                     rhs=aggs[:], start=True, stop=False)
                    nc.tensor.matmul(hp[:], lhsT=w_s["W_root1"][:],
                                     rhs=xT_s[:, sl], start=False, stop=True)
                    nc.scalar.activation(hT[0][:, sl], hp[:], ACTF.Relu)
                    if c < p.WZ1:
                        zp = psum.tile([128, 64], F32, tag="zp", bufs=2)
                        nc.tensor.matmul(zp[:], lhsT=hT[0][:, sl],
                                         rhs=w_s["W_rel2"][:],
                                         start=True, stop=True)
                        zs = spool.tile([128, 64], F32, tag="zs")
                        nc.scalar.activation(zs[:], zp[:], ACTF.Copy)
                        nc.sync.dma_start(z1_own[sl, :], zs[:])

                cdone = 0
                for (toff, nlo, nhi) in L0["groups"]:
                    nt = nlo + nhi
                    G = gpool.tile([128, nt, F], BF16, tag="G0", name="G0")
                    if nlo:
                        nc.gpsimd.dma_gather(
                            G[:, 0:nlo, :], xtab_d[0:min(LO, NCORES * NLOC), :],
                            idx0_s[:, toff * 8:(toff + nlo) * 8],
                            nlo * 128, nlo * 128, F, single_packet=False)
                    if nhi:
                        nc.gpsimd.dma_gather(
                            G[:, nlo:nt, :], xtab_d[LO:NCORES * NLOC, :],
                            idx0_s[:, (toff + nlo) * 8:(toff + nt) * 8],
                            nhi * 128, nhi * 128, F, single_packet=False)
                    selt = selpool.tile([128, nt, 128], BF16, tag="sel0",
                                        name="sel0")
                    build_sel(dv0_s, toff, nt, selt)

                    def Gf(t, toff=toff, G=G):
                        return G[:, t - toff, :]

                    def self0(dvc, toff=toff, selt=selt):
                        return selt[:, dvc - toff, :]

                    while cdone < C1:
                        sch = L0["sched"][cdone]
                        if any(t >= toff + nt for t, _ in sch):
                            break
                        l0_chunk(cdone, Gf, self0)
                        cdone += 1
                        if cdone == p.WZ1:
                            send_layer(1)

                # ---------- layers 2/3 ----------
                def agg_layer(li, hsrcT, CL, wroot, wnext, z_next):
                    Lx = L1 if li == 1 else L2
                    rbuf_d = recv1 if li == 1 else recv2
                    dv_s = dv1_s if li == 1 else dv2_s
                    NTt = Lx["NT"]
                    nr = Lx["S"] // 128
                    R = gpool.tile([128, NTt, 64], BF16, tag=f"R{li}",
                                   name=f"R{li}", bufs=1)
                    for m in range(NCORES):
                        nc.sync.dma_start(R[:, m * nr:(m + 1) * nr, :],
                                          rbuf_d[m])
                    # sel windows built on demand; dvcols are consumed in
                    # ascending order (chunk-major assignment), so 2 buffers
                    # round-robin safely.
                    sel_w = {}

                    def sel_of(dvc, sel_w=sel_w, dv_s=dv_s, li=li):
                        w = dvc // SELB
                        if w not in sel_w:
                            sb = min(SELB, Lx["NDV"] - w * SELB)
                            st = selpool.tile([128, SELB, 128], BF16,
                                              tag=f"selx", name=f"sel{li}")
                            build_sel(dv_s, w * SELB, sb, st)
                            sel_w[w] = st
                        return sel_w[w][:, dvc % SELB, :]

                    for c in range(CL):
                        sl = slice(c * 128, (c + 1) * 128)
                        hp = psum.tile([64, 128], F32, tag="hp", bufs=2)
                        nc.tensor.matmul(hp[:], lhsT=wroot[:],
                                         rhs=hsrcT[:, sl], start=True,
                                         stop=False)
                        sch = Lx["sched"][c]
                        for i, (t, dvc) in enumerate(sch):
                            nc.tensor.matmul(
                                hp[:], lhsT=R[:, t, :], rhs=sel_of(dvc),
                                start=False, stop=(i == len(sch) - 1))
                        if not sch:
                            nc.tensor.matmul(hp[:], lhsT=zero_s[:],
                                             rhs=hsrcT[0:1, sl],
                                             start=False, stop=True)
                        nc.scalar.activation(hT[li][:, sl], hp[:], ACTF.Relu)
                        if z_next is not None:
                            zp = psum.tile([128, 64], F32, tag="zp", bufs=2)
                            nc.tensor.matmul(zp[:], lhsT=hT[li][:, sl],
                                             rhs=wnext[:], start=True,
                                             stop=True)
                            zs = spool.tile([128, 64], F32, tag="zs")
                            nc.scalar.activation(zs[:], zp[:], ACTF.Copy)
                            nc.sync.dma_start(z_next[sl, :], zs[:])

                agg_layer(1, hT[0], C2, w_s["W_root2"], w_s["W_rel3"],
                          z2_own)
                send_layer(2)
                agg_layer(2, hT[1], C3, w_s["W_root3"], None, None)

                # ---------- pool + fc + softmax ----------
                hm = spool.tile([64, C3 * 128], F32, tag="hm")
                nc.vector.tensor_tensor(out=hm[:], in0=hT[2][:],
                                        in1=mask_s[:], op=AX.mult)
                ppart = spool.tile([64, 1], F32, tag="ppart")
                nc.vector.tensor_reduce(ppart[:], hm[:],
                                        axis=mybir.AxisListType.X, op=AX.add)
                nc.sync.dma_start(pool_in[:], ppart[:])
                if not skip_collectives:
                    nc.gpsimd.collective_compute(
                        "AllReduce", AX.add, replica_groups=rg,
                        ins=[pool_in.opt()], outs=[pool_out.opt()])
                pooled = spool.tile([64, 1], F32, tag="pooled")
                nc.sync.dma_start(pooled[:], pool_out[:])
                mean_s = spool.tile([64, 1], F32, tag="mean")
                nc.vector.tensor_scalar_mul(mean_s[:], pooled[:],
                                            1.0 / max(p.n0, 1))
                lg_p = psum.tile([1, 10], F32, tag="lg")
                nc.tensor.matmul(lg_p[:], lhsT=mean_s[:], rhs=wfc_s[:],
                                 start=True, stop=True)
                logits = spool.tile([1, 10], F32, tag="logits")
                nc.vector.tensor_tensor(out=logits[:], in0=lg_p[:],
                                        in1=bfc_s[:], op=AX.add)
                mx = spool.tile([1, 1], F32, tag="mx")
                nc.vector.tensor_reduce(mx[:], logits[:],
                                        axis=mybir.AxisListType.X, op=AX.max)
                nmx = spool.tile([1, 1], F32, tag="nmx")
                nc.vector.tensor_scalar_mul(nmx[:], mx[:], -1.0)
                es = spool.tile([1, 10], F32, tag="es")
                nc.scalar.activation(es[:], logits[:], ACTF.Exp,
                                     bias=nmx[:, 0:1])
                ssum = spool.tile([1, 1], F32, tag="ssum")
                nc.vector.tensor_reduce(ssum[:], es[:],
                                        axis=mybir.AxisListType.X, op=AX.add)
                inv = spool.tile([1, 1], F32, tag="inv")
                nc.vector.reciprocal(inv[:], ssum[:])
                probs_s = spool.tile([1, 10], F32, tag="probs")
                nc.vector.tensor_scalar_mul(probs_s[:], es[:], inv[:, 0:1])
                nc.sync.dma_start(out_d[:], probs_s[:])

    nc.compile()
    return nc


def _in_maps(p, inputs):
    import ml_dtypes
    wb = {k: np.ascontiguousarray(
        np.asarray(inputs[k], np.float32).astype(ml_dtypes.bfloat16))
        for k in ["W_rel1", "W_root1", "W_rel2", "W_root2",
                  "W_rel3", "W_root3"]}
    wfc = np.ascontiguousarray(np.asarray(inputs["W_fc"], np.float32))
    bfc = np.ascontiguousarray(
        np.asarray(inputs["b_fc"], np.float32).reshape(1, 10))
    maps = []
    for k in range(NCORES):
        maps.append({
            "x_tab": p.x_tab, "xT": p.xT[k],
            "idx0": p.idx0[k], "dv0": p.dv0[k],
            "sidx1": p.sidx1[k], "dv1": p.dv1[k],
            "sidx2": p.sidx2[k], "dv2": p.dv2[k],
            "iota": p.iota, "mask": p.mask[k],
            **wb, "W_fc": wfc, "b_fc": bfc,
        })
    return maps


def kernel(**inputs) -> np.ndarray:
    p = build_plan(inputs["x"], inputs["edge_index"], inputs["batch"])
    nc = build_program(p)
    res = run_bass_kernel_spmd(nc, _in_maps(p, inputs), list(range(NCORES)))
    return np.asarray(res.results[0]["probs"]).reshape(10).astype(np.float32)
